# revision 31
# baseline (speedup 1.0000x reference)
"""Trainium2 Bass kernel for nn_Apply_Mask (topk_masking).

Reference semantics, per (batch, channel) slice of shape 32x32:
  - find argmax location (mh, mw)
  - build clipped 5x5 box around it; S = 1 - box
  - lam = 1024 / sum(S)
  - out = T != 0 ? x * S * lam : x

Sharding: embarrassingly data-parallel over the 32768 (b*c) slices;
core i takes slices [4096*i, 4096*(i+1)).

Per-core layout: partition p holds 32 slices [32p, 32p+32) along the free
dim; tile t = slice 32p+t at free offset t*1024.

Math: with sel = (T != 0), a = sel ? lam : 1 and the binary box mask
q2 = (row_in * sel) (x) col_in, the output is
    out = a * u,  u = (q2 == 0) ? x : 0.
The (q2==0)*x select uses a scalar *immediate*, so it batches 4 tiles per
scalar_tensor_tensor; the per-tile a (and the f32->bf16 downcast) ride the
ScalarE activation copy for free.

Phase structure (DVE pipeline drains stall a dependent consumer ~1us, so
the serial per-slice scalar chain runs ONCE over all 32 slices, not per
group):
  1. argmax for all tiles: per-tile max8, 4-tile-batched find_index8
  2. one merged per-slice scalar chain ([P,32])
  3. box masks (compares on DVE, arithmetic on Pool)
  4. per 4-tile batch: Pool outer product -> DVE batched select ->
     ScalarE scaled bf16 downcast -> output DMA

Engine split: DVE argmax/compares/select; Pool mask arithmetic + outers;
ScalarE out = bf16(a*u); DMA f32 in (16 MiB), bf16 out (8 MiB).
"""
import sys

for _p in ("/opt/trn_rl_repo",):
    if _p not in sys.path:
        sys.path.insert(0, _p)

import numpy as np

import concourse.bass as bass
import concourse.tile as tile
from concourse import bacc, mybir
from concourse.bass_utils import run_bass_kernel_spmd

P = 128          # partitions
NT = 32          # tiles (slices) per partition
H = W = 32
HW = H * W
N_CORES = 8
SLICES_PER_CORE = P * NT  # 4096

OUT_BF16 = True   # downcast output to bf16 on ScalarE (rel err ~2e-3)
KQ = 4            # tiles per outer-product / select batch (= DMA chunk)

f32 = mybir.dt.float32
bf16 = mybir.dt.bfloat16
u16 = mybir.dt.uint16
Alu = mybir.AluOpType
Act = mybir.ActivationFunctionType

_cached = {}


def _build(half: int):
    odt = bf16 if OUT_BF16 else f32
    NB = NT // KQ          # batches

    nc = bacc.Bacc("TRN2", target_bir_lowering=False, debug=False,
                   num_devices=N_CORES)
    x_in = nc.dram_tensor("x", [P, NT * HW], f32, kind="ExternalInput").ap()
    sel_in = nc.dram_tensor("sel", [P, NT], f32, kind="ExternalInput").ap()
    io_in = nc.dram_tensor("io32", [P, 32], f32, kind="ExternalInput").ap()
    out_d = nc.dram_tensor("out", [P, NT * HW], odt, kind="ExternalOutput").ap()

    with tile.TileContext(nc) as tc:
        from contextlib import ExitStack
        with ExitStack() as ctx:
            xpool = ctx.enter_context(tc.tile_pool(name="xp", bufs=1))
            mid = ctx.enter_context(tc.tile_pool(name="mid", bufs=1))
            small = ctx.enter_context(tc.tile_pool(name="small", bufs=1))
            qpool = ctx.enter_context(tc.tile_pool(name="qp", bufs=2))
            opool = ctx.enter_context(tc.tile_pool(name="op", bufs=2))

            # ---- input DMA, one chunk per KQ-tile batch ----
            xc = []
            for b_ in range(NB):
                t_ = xpool.tile([P, KQ * HW], f32, name=f"x{b_}", tag=f"x{b_}")
                nc.sync.dma_start(t_[:], x_in[:, b_ * KQ * HW:(b_ + 1) * KQ * HW])
                xc.append(t_)

            def x_tile(t):
                return xc[t // KQ][:, (t % KQ) * HW:(t % KQ + 1) * HW]

            selp = small.tile([P, NT], f32)
            nc.sync.dma_start(selp[:], sel_in)
            io32 = small.tile([P, 32], f32)
            nc.sync.dma_start(io32[:], io_in)

            max8 = mid.tile([P, NT, 8], f32)
            idx8 = mid.tile([P, NB, 8], u16)   # batched indices (0..KQ*HW)

            # ---- phase 1: argmax for all tiles ----
            for b_ in range(NB):
                for j in range(KQ):
                    t = b_ * KQ + j
                    nc.vector.max(max8[:, t], x_tile(t))
                # in_max = [m0,s0,m1,s1,...] for the KQ tiles of batch b_
                inm = small.tile([P, 2 * KQ], f32, name=f"inm{b_}", tag="inm")
                nc.vector.tensor_copy(
                    inm[:].rearrange("p (t k) -> p t k", t=KQ, k=2),
                    max8[:, b_ * KQ:(b_ + 1) * KQ, 0:2])
                nc.vector.max_index(idx8[:, b_], inm[:], xc[b_][:])

            # ---- phase 2: merged per-slice scalar chain ([P, NT]) ----
            def sm(name, dt=f32):
                return small.tile([P, NT], dt, name=name, tag=name)

            idx_u = sm("idxu", u16)
            nc.vector.tensor_copy(
                idx_u[:].rearrange("p (b j) -> p b j", b=NB, j=KQ),
                idx8[:, :, 0:2 * KQ:2])
            mh_u = sm("mhu", u16)
            mw_u = sm("mwu", u16)
            nc.vector.tensor_scalar(mh_u[:], idx_u[:], 5, 31,
                                    Alu.logical_shift_right, Alu.bitwise_and)
            nc.vector.tensor_scalar(mw_u[:], idx_u[:], 31, None, Alu.bitwise_and)
            mh = sm("mh"); mw = sm("mw")
            nc.vector.tensor_copy(mh[:], mh_u[:])
            nc.vector.tensor_copy(mw[:], mw_u[:])
            h1 = sm("h1"); h2 = sm("h2"); w1 = sm("w1"); w2 = sm("w2")
            nc.vector.tensor_scalar(h1[:], mh[:], float(half), 0.0, Alu.subtract, Alu.max)
            nc.vector.tensor_scalar(w1[:], mw[:], float(half), 0.0, Alu.subtract, Alu.max)
            nc.vector.tensor_scalar(h2[:], mh[:], float(half), float(H - 1), Alu.add, Alu.min)
            nc.vector.tensor_scalar(w2[:], mw[:], float(half), float(W - 1), Alu.add, Alu.min)

            # ---- phase 3: masks (emitted mid-chain: the big compares fill
            # the drain stalls of the dependent scalar-chain tail) ----
            io_b = io32[:, None, :].broadcast_to([P, NT, 32])
            col_in = mid.tile([P, NT, W], f32)
            col_gt = mid.tile([P, NT, W], f32)
            row_sl = mid.tile([P, NT, H], f32)
            row_gt = mid.tile([P, NT, H], f32)
            rl1 = sm("rl1"); cl1 = sm("cl1"); area = sm("area")
            denom = sm("denom"); recip = sm("recip"); lam1 = sm("lam1"); a_t = sm("a")
            nc.vector.tensor_tensor(col_in[:], io_b, w1[:, :, None].broadcast_to([P, NT, W]), Alu.is_ge)
            nc.vector.scalar_tensor_tensor(rl1[:], h2[:], 1.0, h1[:], Alu.add, Alu.subtract)
            nc.vector.tensor_tensor(col_gt[:], io_b, w2[:, :, None].broadcast_to([P, NT, W]), Alu.is_gt)
            nc.vector.scalar_tensor_tensor(cl1[:], w2[:], 1.0, w1[:], Alu.add, Alu.subtract)
            nc.vector.tensor_tensor(row_sl[:], io_b, h1[:, :, None].broadcast_to([P, NT, H]), Alu.is_ge)
            nc.vector.tensor_tensor(area[:], rl1[:], cl1[:], Alu.mult)
            nc.vector.tensor_tensor(row_gt[:], io_b, h2[:, :, None].broadcast_to([P, NT, H]), Alu.is_gt)
            nc.vector.tensor_scalar(denom[:], area[:], -1.0, float(HW), Alu.mult, Alu.add)
            nc.gpsimd.tensor_tensor(col_in[:], col_in[:], col_gt[:], Alu.subtract)
            nc.vector.reciprocal(recip[:], denom[:])
            nc.gpsimd.tensor_tensor(row_sl[:], row_sl[:], row_gt[:], Alu.subtract)
            nc.vector.tensor_scalar(lam1[:], recip[:], float(HW), -1.0, Alu.mult, Alu.add)
            nc.gpsimd.tensor_tensor(row_sl[:], row_sl[:], selp[:, :, None].broadcast_to([P, NT, H]), Alu.mult)
            nc.vector.scalar_tensor_tensor(a_t[:], lam1[:], 0.0, selp[:], Alu.add, Alu.mult)
            nc.vector.tensor_scalar(a_t[:], a_t[:], 1.0, None, Alu.add)

            # ---- phase 4: outer -> select (DVE) -> scale+cast (ACT) ----
            # First DVE_Q outer batches on DVE (both emitted before their
            # selects to dodge the producer->consumer drain stall) so selects
            # start immediately; Pool produces the later batches concurrently.
            DVE_Q = 2

            def outer(b_, q):
                tb = b_ * KQ
                q_eng = nc.vector if b_ < DVE_Q else nc.gpsimd
                q_eng.tensor_tensor(
                    q[:],
                    row_sl[:, tb:tb + KQ, :, None].broadcast_to([P, KQ, H, W]),
                    col_in[:, tb:tb + KQ, None, :].broadcast_to([P, KQ, H, W]),
                    Alu.mult,
                )

            qb = {}
            for b_ in range(min(DVE_Q, NB)):
                qb[b_] = qpool.tile([P, KQ, H, W], f32, name=f"q{b_}", tag="q")
                outer(b_, qb[b_])

            for b_ in range(NB):
                tb = b_ * KQ
                if b_ in qb:
                    q = qb[b_]
                else:
                    q = qpool.tile([P, KQ, H, W], f32, name=f"q{b_}", tag="q")
                    outer(b_, q)
                # u = (q2 == 0) * x, batched over KQ tiles, in place over q
                nc.vector.scalar_tensor_tensor(
                    q[:], q[:], 0.0,
                    xc[b_][:].rearrange("p (t h w) -> p t h w", t=KQ, h=H, w=W),
                    Alu.is_equal, Alu.mult,
                )
                # out = bf16(a * u) on ScalarE, then DMA the chunk
                o_c = opool.tile([P, KQ * HW], odt, name=f"o{b_}", tag="oc")
                for j in range(KQ):
                    t = tb + j
                    nc.scalar.activation(
                        o_c[:, j * HW:(j + 1) * HW],
                        q[:, j].rearrange("p h w -> p (h w)"),
                        Act.Copy, bias=0.0, scale=a_t[:, t, None],
                    )
                nc.sync.dma_start(out_d[:, tb * HW:(tb + KQ) * HW], o_c[:])

    nc.compile()
    return nc


def _get_nc(half: int):
    if half not in _cached:
        _cached[half] = _build(half)
    return _cached[half]


def _shard_inputs(x, T):
    xf = np.ascontiguousarray(x, dtype=np.float32).reshape(-1, HW)   # [32768, 1024]
    sel = (np.asarray(T).reshape(-1) != 0).astype(np.float32)        # [32768]
    io32 = np.tile(np.arange(32, dtype=np.float32), (P, 1))
    in_maps = []
    for i in range(N_CORES):
        lo = i * SLICES_PER_CORE
        hi = lo + SLICES_PER_CORE
        in_maps.append({
            "x": np.ascontiguousarray(xf[lo:hi].reshape(P, NT * HW)),
            "sel": np.ascontiguousarray(sel[lo:hi].reshape(P, NT)),
            "io32": io32,
        })
    return in_maps


def run(inputs, trace=False, **kw):
    x = inputs["x"]
    T = inputs["T"]
    drop_block = int(np.asarray(inputs["drop_block"]))
    half = drop_block // 2
    b, c, h, w = x.shape
    assert (h, w) == (H, W) and b * c == N_CORES * SLICES_PER_CORE, \
        f"kernel hardcoded for (128,256,32,32); got {x.shape}"

    nc = _get_nc(half)
    in_maps = _shard_inputs(x, T)
    res = run_bass_kernel_spmd(nc, in_maps, core_ids=list(range(N_CORES)),
                               trace=trace, **kw)
    parts = [np.asarray(res.results[i]["out"]).astype(np.float32)
              .reshape(SLICES_PER_CORE, HW)
             for i in range(N_CORES)]
    out = np.concatenate(parts, axis=0).reshape(b, c, h, w)
    return out, res


def kernel(**inputs) -> np.ndarray:
    out, _ = run(inputs, trace=False)
    return out


# revision 32
# speedup vs baseline: 1.0067x; 1.0067x over previous
"""Trainium2 Bass kernel for nn_Apply_Mask (topk_masking).

Per (batch, channel) slice of shape 32x32: find the argmax location, build
a clipped 5x5 box around it, S = 1 - box, lam = 1024/sum(S), and
out = (T != 0) ? x * S * lam : x.

Sharding: data-parallel over the 32768 b*c slices; core i takes slices
[4096*i, 4096*(i+1)). Per-core layout: partition p holds slices
[32p, 32p+32); tile t = slice 32p+t at free offset t*1024.

Math: with sel = (T != 0), a = sel ? lam : 1 and the binary box mask
q2 = (row_in * sel) (x) col_in, the output is
    out = a * u,  u = (q2 == 0) ? x : 0.
The select uses a scalar immediate so it batches 4 tiles per
scalar_tensor_tensor; the per-tile a and the f32->bf16 downcast ride the
ScalarE activation copy.

Engine split: DVE does the exact f32 argmax (per-tile max8 + 4-tile
batched find_index8), iota compares, per-slice scalar math, and the
batched select; GpSimd does mask arithmetic + outer products; ScalarE
does out = bf16(a*u); DMA moves f32 in (16 MiB), bf16 out (8 MiB).
Tiles are processed in 4 groups so group g's mask/apply overlaps group
g+1's argmax. Measured ~200us/core on TRN2 (memory roofline ~93us),
global rel err 1.7e-3 (bf16 output only; compute is bit-exact f32).
"""
import sys

for _p in ("/opt/trn_rl_repo",):
    if _p not in sys.path:
        sys.path.insert(0, _p)

import numpy as np

import concourse.bass as bass
import concourse.tile as tile
from concourse import bacc, mybir
from concourse.bass_utils import run_bass_kernel_spmd

P = 128
NT = 32
H = W = 32
HW = H * W
N_CORES = 8
SLICES_PER_CORE = P * NT

OUT_BF16 = True
KQ = 4
NGROUP = 4

f32 = mybir.dt.float32
bf16 = mybir.dt.bfloat16
u16 = mybir.dt.uint16
Alu = mybir.AluOpType
Act = mybir.ActivationFunctionType

_cached = {}


def _build(half: int):
    odt = bf16 if OUT_BF16 else f32
    GT = NT // NGROUP
    NB = NT // KQ

    nc = bacc.Bacc("TRN2", target_bir_lowering=False, debug=False,
                   num_devices=N_CORES)
    x_in = nc.dram_tensor("x", [P, NT * HW], f32, kind="ExternalInput").ap()
    sel_in = nc.dram_tensor("sel", [P, NT], f32, kind="ExternalInput").ap()
    io_in = nc.dram_tensor("io32", [P, 32], f32, kind="ExternalInput").ap()
    out_d = nc.dram_tensor("out", [P, NT * HW], odt, kind="ExternalOutput").ap()

    with tile.TileContext(nc) as tc:
        from contextlib import ExitStack
        with ExitStack() as ctx:
            xpool = ctx.enter_context(tc.tile_pool(name="xp", bufs=1))
            mid = ctx.enter_context(tc.tile_pool(name="mid", bufs=1))
            small = ctx.enter_context(tc.tile_pool(name="small", bufs=1))
            qpool = ctx.enter_context(tc.tile_pool(name="qp", bufs=2))
            opool = ctx.enter_context(tc.tile_pool(name="op", bufs=2))

            xc = []
            for b_ in range(NB):
                t_ = xpool.tile([P, KQ * HW], f32, name=f"x{b_}", tag=f"x{b_}")
                nc.sync.dma_start(t_[:], x_in[:, b_ * KQ * HW:(b_ + 1) * KQ * HW])
                xc.append(t_)

            def x_tile(t):
                return xc[t // KQ][:, (t % KQ) * HW:(t % KQ + 1) * HW]

            selp = small.tile([P, NT], f32)
            nc.sync.dma_start(selp[:], sel_in)
            io32 = small.tile([P, 32], f32)
            nc.sync.dma_start(io32[:], io_in)

            max8 = mid.tile([P, NT, 8], f32)
            idx8 = mid.tile([P, NB, 8], u16)
            col_in = mid.tile([P, NT, W], f32)
            col_gt = mid.tile([P, NT, W], f32)
            row_sl = mid.tile([P, NT, H], f32)
            row_gt = mid.tile([P, NT, H], f32)
            io_b = io32[:, None, :]

            def smalls(name, dt=f32):
                return [small.tile([P, GT], dt, name=f"{name}{g}", tag=f"{name}{g}")
                        for g in range(NGROUP)]

            idx_u = smalls("idxu", u16)
            mh_u = smalls("mhu", u16)
            mw_u = smalls("mwu", u16)
            mh = smalls("mh"); mw = smalls("mw")
            h1 = smalls("h1"); h2 = smalls("h2"); w1 = smalls("w1"); w2 = smalls("w2")
            rl = smalls("rl"); cl1 = smalls("cl1"); area = smalls("area")
            denom = smalls("denom"); recip = smalls("recip"); lam1 = smalls("lam1")
            a_t = smalls("a")

            for g in range(NGROUP):
                gl = g * GT
                gsl = slice(gl, gl + GT)
                b0 = gl // KQ
                nbg = GT // KQ

                for t in range(gl, gl + GT):
                    nc.vector.max(max8[:, t], x_tile(t))
                for b_ in range(b0, b0 + nbg):
                    inm = small.tile([P, 2 * KQ], f32, name=f"inm{b_}", tag="inm")
                    nc.vector.tensor_copy(
                        inm[:].rearrange("p (t k) -> p t k", t=KQ, k=2),
                        max8[:, b_ * KQ:(b_ + 1) * KQ, 0:2])
                    nc.vector.max_index(idx8[:, b_], inm[:], xc[b_][:])

                nc.vector.tensor_copy(
                    idx_u[g][:].rearrange("p (b j) -> p b j", b=nbg, j=KQ),
                    idx8[:, b0:b0 + nbg, 0:2 * KQ:2])
                nc.vector.tensor_scalar(mh_u[g][:], idx_u[g][:], 5, 31,
                                        Alu.logical_shift_right, Alu.bitwise_and)
                nc.vector.tensor_scalar(mw_u[g][:], idx_u[g][:], 31, None, Alu.bitwise_and)
                nc.vector.tensor_copy(mh[g][:], mh_u[g][:])
                nc.vector.tensor_copy(mw[g][:], mw_u[g][:])
                nc.vector.tensor_scalar(h1[g][:], mh[g][:], float(half), 0.0, Alu.subtract, Alu.max)
                nc.vector.tensor_scalar(h2[g][:], mh[g][:], float(half), float(H - 1), Alu.add, Alu.min)
                nc.vector.tensor_scalar(w1[g][:], mw[g][:], float(half), 0.0, Alu.subtract, Alu.max)
                nc.vector.tensor_scalar(w2[g][:], mw[g][:], float(half), float(W - 1), Alu.add, Alu.min)
                nc.vector.tensor_tensor(rl[g][:], h2[g][:], h1[g][:], Alu.subtract)
                nc.vector.tensor_tensor(cl1[g][:], w2[g][:], w1[g][:], Alu.subtract)
                nc.vector.tensor_scalar(cl1[g][:], cl1[g][:], 1.0, None, Alu.add)
                nc.vector.scalar_tensor_tensor(area[g][:], rl[g][:], 1.0, cl1[g][:], Alu.add, Alu.mult)
                nc.vector.tensor_scalar(denom[g][:], area[g][:], -1.0, float(HW), Alu.mult, Alu.add)
                nc.vector.reciprocal(recip[g][:], denom[g][:])
                nc.vector.tensor_scalar(lam1[g][:], recip[g][:], float(HW), -1.0, Alu.mult, Alu.add)
                nc.vector.scalar_tensor_tensor(a_t[g][:], lam1[g][:], 0.0, selp[:, gsl], Alu.add, Alu.mult)
                nc.vector.tensor_scalar(a_t[g][:], a_t[g][:], 1.0, None, Alu.add)

                iog = io_b.broadcast_to([P, GT, 32])
                nc.vector.tensor_tensor(col_in[:, gsl], iog, w1[g][:, :, None].broadcast_to([P, GT, W]), Alu.is_ge)
                nc.vector.tensor_tensor(col_gt[:, gsl], iog, w2[g][:, :, None].broadcast_to([P, GT, W]), Alu.is_gt)
                nc.gpsimd.tensor_tensor(col_in[:, gsl], col_in[:, gsl], col_gt[:, gsl], Alu.subtract)
                nc.vector.tensor_tensor(row_sl[:, gsl], iog, h1[g][:, :, None].broadcast_to([P, GT, H]), Alu.is_ge)
                nc.vector.tensor_tensor(row_gt[:, gsl], iog, h2[g][:, :, None].broadcast_to([P, GT, H]), Alu.is_gt)
                nc.gpsimd.tensor_tensor(row_sl[:, gsl], row_sl[:, gsl], row_gt[:, gsl], Alu.subtract)
                nc.gpsimd.tensor_tensor(row_sl[:, gsl], row_sl[:, gsl], selp[:, gsl, None].broadcast_to([P, GT, H]), Alu.mult)

                for b_ in range(b0, b0 + nbg):
                    tb = b_ * KQ
                    q = qpool.tile([P, KQ, H, W], f32, name=f"q{b_}", tag="q")
                    nc.gpsimd.tensor_tensor(
                        q[:],
                        row_sl[:, tb:tb + KQ, :, None].broadcast_to([P, KQ, H, W]),
                        col_in[:, tb:tb + KQ, None, :].broadcast_to([P, KQ, H, W]),
                        Alu.mult,
                    )
                    nc.vector.scalar_tensor_tensor(
                        q[:], q[:], 0.0,
                        xc[b_][:].rearrange("p (t h w) -> p t h w", t=KQ, h=H, w=W),
                        Alu.is_equal, Alu.mult,
                    )
                    o_c = opool.tile([P, KQ * HW], odt, name=f"o{b_}", tag="oc")
                    for j in range(KQ):
                        t = tb + j
                        nc.scalar.activation(
                            o_c[:, j * HW:(j + 1) * HW],
                            q[:, j].rearrange("p h w -> p (h w)"),
                            Act.Copy, bias=0.0, scale=a_t[g][:, t - gl, None],
                        )
                    nc.sync.dma_start(out_d[:, tb * HW:(tb + KQ) * HW], o_c[:])

    nc.compile()
    return nc


def _get_nc(half: int):
    if half not in _cached:
        _cached[half] = _build(half)
    return _cached[half]


def _shard_inputs(x, T):
    xf = np.ascontiguousarray(x, dtype=np.float32).reshape(-1, HW)
    sel = (np.asarray(T).reshape(-1) != 0).astype(np.float32)
    io32 = np.tile(np.arange(32, dtype=np.float32), (P, 1))
    in_maps = []
    for i in range(N_CORES):
        lo = i * SLICES_PER_CORE
        hi = lo + SLICES_PER_CORE
        in_maps.append({
            "x": np.ascontiguousarray(xf[lo:hi].reshape(P, NT * HW)),
            "sel": np.ascontiguousarray(sel[lo:hi].reshape(P, NT)),
            "io32": io32,
        })
    return in_maps


def run(inputs, trace=False, **kw):
    x = inputs["x"]
    T = inputs["T"]
    drop_block = int(np.asarray(inputs["drop_block"]))
    half = drop_block // 2
    b, c, h, w = x.shape
    assert (h, w) == (H, W) and b * c == N_CORES * SLICES_PER_CORE, \
        f"kernel hardcoded for (128,256,32,32); got {x.shape}"

    nc = _get_nc(half)
    in_maps = _shard_inputs(x, T)
    res = run_bass_kernel_spmd(nc, in_maps, core_ids=list(range(N_CORES)),
                               trace=trace, **kw)
    parts = [np.asarray(res.results[i]["out"]).astype(np.float32)
              .reshape(SLICES_PER_CORE, HW)
             for i in range(N_CORES)]
    out = np.concatenate(parts, axis=0).reshape(b, c, h, w)
    return out, res


def kernel(**inputs) -> np.ndarray:
    out, _ = run(inputs, trace=False)
    return out
